# revision 1
# baseline (speedup 1.0000x reference)
"""ChildSum TreeLSTM (N=8192 nodes, 4-ary static heap tree, H=256, D=300) on 8 trn2 NeuronCores.

Strategy
--------
The tree is static: node i's children are 4i+1..4i+4 (clipped at N). The reverse
scan (children before parents) is equivalent to processing the tree level by
level, bottom-up; nodes within a level are independent, so each level is a
batched LSTM cell (matmuls + elementwise).

Sharding: the 256 level-4 subtrees are partitioned across the 8 cores (balanced
by the number of *internal* level-6 descendants, which determines level-7 leaf
count). Each core processes its forest fully locally — children of a sorted node
range are contiguous in the next level's sorted array, so the recurrence needs
no gathers and no cross-core communication. Cores output their 32 level-4 root
(h, c) states; the tiny top of the tree (levels 3..0, 85 nodes) plus the final
log_softmax run on the host in numpy.

On-device layout: everything is transposed — feature dim on SBUF partitions
(256 features = 2 halves of 128), nodes along the free axis. The child-h sums
and per-child forget gates then become strided slicing along the free axis, and
the x-side / h-side gate projections accumulate into the same PSUM tile.
Biases (bx + bh, zeros in practice) are folded into an extra ones-row of the
x-side matmul, so pad columns (zero x) self-compute to h = c = 0.
"""

import numpy as np
import ml_dtypes

BF16 = ml_dtypes.bfloat16

N = 8192
H = 256
D = 300
K = 4
OUT = 4
NCORES = 8
L7P = 384           # padded level-7 columns per core (4 * IPMAX)
IPMAX = 96          # max internal level-6 nodes per core
KDIM = 304          # padded contraction rows of xt/wx (300 emb + 1 ones + pad)
KUSE = 301          # rows actually used in matmuls
XCOLS = L7P + 512 + 128 + 32   # 1056 per-core node columns: [L7 | L6 | L5 | L4]

GATE_MAP = [0, 2, 3, 1]  # our gate order [i, o, u, f] -> reference gate indices

F32 = np.float32


def _build_plan():
    """Assign the 256 level-4 subtrees to 8 cores; build per-core column maps."""
    # w(u) = number of internal (has-children) level-6 descendants of L4 node u.
    # Full-weight subtrees (w=16) are u in [85, 127); u=127 has w=11; rest 0.
    full = list(range(85, 127))                               # 42 subtrees
    lights = list(range(128, 341))                            # 213 subtrees
    heavy_counts = [6, 6, 5, 5, 5, 5, 5, 5]                   # sums to 42
    light_counts = [26, 26, 26, 27, 27, 27, 27, 27]           # sums to 213
    cores = []
    hpos = 0
    lpos = 0
    for c in range(NCORES):
        hs = full[hpos:hpos + heavy_counts[c]]
        hpos += heavy_counts[c]
        if c == 2:
            hs = hs + [127]                                   # w sums: 96,96,91,80*5
        ls = lights[lpos:lpos + light_counts[c]]
        lpos += light_counts[c]
        cores.append(sorted(hs + ls))
    all_l4 = sorted(u for cs in cores for u in cs)
    assert all_l4 == list(range(85, 341)), "L4 assignment must partition [85, 341)"

    plan = []
    for c in range(NCORES):
        l4 = cores[c]
        assert len(l4) == 32
        l5 = [4 * u + 1 + k for u in l4 for k in range(K)]
        l6 = [4 * v + 1 + k for v in l5 for k in range(K)]
        wc = sum(1 for x in l6 if x < 2048)
        assert wc <= IPMAX
        l7 = []
        for x in l6[:wc]:
            for k in range(K):
                ch = 4 * x + 1 + k
                l7.append(ch if ch < N else -1)
        l7 += [-1] * (L7P - len(l7))
        cols = np.array(l7 + l6 + l5 + l4, dtype=np.int64)
        assert cols.shape == (XCOLS,)
        plan.append((cols, wc, np.array(l4, dtype=np.int64)))
    return plan


_PLAN = _build_plan()

# chunk schedule: (xoff, ncols, ip, child_level, child_col_off, out_level, out_off)
# child/out levels refer to state buffers keyed 7, 6, 5, 4. L5 is split in two
# so its halves chain off the two independent L6 chunks — the scheduler can
# overlap one chain's elementwise tail with the other chain's matmuls.
_CHUNKS = [
    (0,    256, 0,   None, 0,   7, 0),     # L7 leaves, part A
    (256,  128, 0,   None, 0,   7, 256),   # L7 leaves, part B
    (640,  256, 0,   None, 0,   6, 256),   # L6 leaf-only half
    (384,  256, 96,  7,    0,   6, 0),     # L6 internal half
    (896,  128, 128, 6,    0,   5, 0),     # L5
    (1024, 32,  32,  5,    0,   4, 0),     # L4
]
_STATE_COLS = {7: L7P, 6: 512, 5: 128, 4: 32}


def _static_tree():
    idx = np.arange(N)[:, None] * K + 1 + np.arange(K)[None, :]
    mask = (idx < N).astype(F32)
    idx = np.where(idx < N, idx, 0).astype(np.int32)
    return idx, mask


_STATIC_IDX, _STATIC_MASK = _static_tree()


def _pack_weights(Wx, bx, Wh, bh):
    wx = np.zeros((KDIM, 4 * H), dtype=F32)  # cast to bf16 at return
    for g, rg in enumerate(GATE_MAP):
        wx[:D, H * g:H * (g + 1)] = np.asarray(Wx[rg], dtype=F32).T
        wx[D, H * g:H * (g + 1)] = np.asarray(bx[rg], dtype=F32) + np.asarray(bh[rg], dtype=F32)
    wh = np.zeros((H, 3 * H), dtype=F32)
    for g, rg in enumerate([0, 2, 3]):  # i, o, u
        wh[:, H * g:H * (g + 1)] = np.asarray(Wh[rg], dtype=F32).T
    whf = np.ascontiguousarray(np.asarray(Wh[1], dtype=F32).T)
    return wx.astype(BF16), wh, whf


def _pack_xt(xs, emb_table):
    X = np.asarray(emb_table, dtype=F32)[np.asarray(xs)]
    xts = []
    for cols, _, _ in _PLAN:
        xt = np.zeros((KDIM, XCOLS), dtype=F32)
        real = cols >= 0
        xt[:D, real] = X[cols[real]].T
        xt[D, real] = 1.0
        xts.append(xt.astype(BF16))
    return xts


def _sigmoid(x):
    return (1.0 / (1.0 + np.exp(-x))).astype(F32)


def _host_top(Hbuf, Cbuf, xs, emb_table, Wx, bx, Wh, bh):
    """Compute tree levels 3..0 (nodes 0..84) on the host, numpy fp32."""
    Wx = np.asarray(Wx, dtype=F32)
    bx = np.asarray(bx, dtype=F32)
    Wh = np.asarray(Wh, dtype=F32)
    bh = np.asarray(bh, dtype=F32)
    emb = np.asarray(emb_table, dtype=F32)
    xs = np.asarray(xs)
    for lo, hi in [(21, 85), (5, 21), (1, 5), (0, 1)]:
        ids = np.arange(lo, hi)
        Xl = emb[xs[ids]]                                   # [n, D]
        gx = np.einsum('ghd,nd->ngh', Wx, Xl).astype(F32) + bx
        cidx = ids[:, None] * K + 1 + np.arange(K)[None, :]  # all valid (< 341)
        Hc = Hbuf[cidx]
        Cc = Cbuf[cidx]
        hs = Hc.sum(1)
        ig = _sigmoid(gx[:, 0] + hs @ Wh[0].T + bh[0])
        og = _sigmoid(gx[:, 2] + hs @ Wh[2].T + bh[2])
        ug = np.tanh(gx[:, 3] + hs @ Wh[3].T + bh[3]).astype(F32)
        f = _sigmoid(gx[:, 1][:, None, :] + Hc @ Wh[1].T + bh[1])
        cc = ig * ug + (f * Cc).sum(1)
        hh = og * np.tanh(cc).astype(F32)
        Hbuf[ids] = hh
        Cbuf[ids] = cc
    return Hbuf[0]


def _log_softmax(x):
    m = np.max(x)
    e = np.exp(x - m)
    return (x - m - np.log(e.sum())).astype(F32)


def simulate_cores_numpy(inputs):
    """Numpy emulation of the exact device data layout & chunk schedule.

    Returns (Hbuf, Cbuf) filled for nodes [85, 341) — for validating the plan
    against the reference without hardware.
    """
    xs = np.asarray(inputs["xs"])
    wx, wh, whf = _pack_weights(inputs["Wx"], inputs["bx"], inputs["Wh"], inputs["bh"])
    xts = _pack_xt(xs, inputs["emb_table"])
    Hbuf = np.zeros((341, H), dtype=F32)
    Cbuf = np.zeros((341, H), dtype=F32)
    for c in range(NCORES):
        cols, wc, l4 = _PLAN[c]
        xt = xts[c]
        state_h = {lv: np.zeros((H, n), dtype=F32) for lv, n in _STATE_COLS.items()}
        state_c = {lv: np.zeros((H, n), dtype=F32) for lv, n in _STATE_COLS.items()}
        for (xoff, nc_, ip, child, coff, outlv, ooff) in _CHUNKS:
            xk = xt[:KUSE, xoff:xoff + nc_].astype(F32)         # [301, nc]
            G = wx[:KUSE].astype(F32).T @ xk                    # [1024, nc]
            gi = G[0:H]
            go = G[H:2 * H]
            gu = G[2 * H:3 * H]
            gf = G[3 * H:4 * H]
            if ip > 0:
                ch_h = state_h[child][:, coff:coff + 4 * ip]    # [H, 4ip]
                ch_c = state_c[child][:, coff:coff + 4 * ip]
                hs = ch_h.reshape(H, ip, K).sum(axis=2)         # [H, ip]
                A = wh.T @ hs                                   # [768, ip]
                gi[:, :ip] += A[0:H]
                go[:, :ip] += A[H:2 * H]
                gu[:, :ip] += A[2 * H:3 * H]
                Fp = whf.T @ ch_h                               # [H, 4ip]
                FA = Fp + np.repeat(gf[:, :ip], K, axis=1)
                FS = _sigmoid(FA) * ch_c
                csum = FS.reshape(H, ip, K).sum(axis=2)
            ig = _sigmoid(gi)
            og = _sigmoid(go)
            ug = np.tanh(gu).astype(F32)
            cc = ig * ug
            if ip > 0:
                cc[:, :ip] += csum
            hh = og * np.tanh(cc).astype(F32)
            state_h[outlv][:, ooff:ooff + nc_] = hh
            state_c[outlv][:, ooff:ooff + nc_] = cc
        Hbuf[l4] = state_h[4].T
        Cbuf[l4] = state_c[4].T
    return Hbuf, Cbuf


# ----------------------------------------------------------------------------
# Bass device program
# ----------------------------------------------------------------------------

_COMPILED = None


def _build_device_program():
    import contextlib

    import concourse.bacc as bacc
    import concourse.tile as tile
    import concourse.mybir as mybir

    f32 = mybir.dt.float32
    f32r = mybir.dt.float32r
    bf16 = mybir.dt.bfloat16
    Sig = mybir.ActivationFunctionType.Sigmoid
    Tanh = mybir.ActivationFunctionType.Tanh

    nc = bacc.Bacc("TRN2", target_bir_lowering=False, debug=False,
                   num_devices=NCORES)

    def mm(out, lhsT, rhs, **kw):
        # float32r operands: same fp32 bytes, single-pass reduced-precision
        # multiply (vs fp32's two half-speed passes + double weight load).
        nc.tensor.matmul(out, lhsT, rhs, **kw)

    xt_d = nc.dram_tensor("xt", [KDIM, XCOLS], bf16, kind="ExternalInput")
    wx_d = nc.dram_tensor("wx", [KDIM, 4 * H], bf16, kind="ExternalInput")
    wh_d = nc.dram_tensor("wh", [H, 3 * H], f32r, kind="ExternalInput")
    whf_d = nc.dram_tensor("whf", [H, H], f32r, kind="ExternalInput")
    out_h_d = nc.dram_tensor("out_h", [128, 2, 32], f32r, kind="ExternalOutput")
    out_c_d = nc.dram_tensor("out_c", [128, 2, 32], f32, kind="ExternalOutput")

    krows = [(0, 128), (128, 256), (256, KUSE)]
    RANGES = [(0, 512), (512, 1024), (1024, XCOLS)]

    with tile.TileContext(nc) as tc:
        with contextlib.ExitStack() as ctx:
            inp = ctx.enter_context(tc.tile_pool(name="inp", bufs=1))
            st = ctx.enter_context(tc.tile_pool(name="state", bufs=1))
            wk = ctx.enter_context(tc.tile_pool(name="work", bufs=2))
            fwk = ctx.enter_context(tc.tile_pool(name="fwork", bufs=3))
            ps = ctx.enter_context(
                tc.tile_pool(name="psum", bufs=2, space="PSUM"))

            # --- inputs to SBUF, spread across otherwise-idle engines; xt is
            # split into chunk-aligned column ranges so the first level can
            # start as soon as its columns land.
            xt_s = []
            wx_s = []
            for k, (r0, r1) in enumerate(krows[:2] + [(256, 304)]):
                t = inp.tile([r1 - r0, 4 * H], bf16, tag=f"wx{k}", name=f"wx{k}")
                wx_s.append(t)
            # wx on gpsimd in column quarters, k-interleaved, so the first
            # gate's weights land quickly
            for q in range(4):
                for k, (r0, r1) in enumerate(krows[:2] + [(256, 304)]):
                    nc.gpsimd.dma_start(
                        out=wx_s[k][:, 256 * q:256 * (q + 1)],
                        in_=wx_d[r0:r1, 256 * q:256 * (q + 1)])
            wh_s = []
            whf_s = []
            for k, (r0, r1) in enumerate([(0, 128), (128, 256)]):
                t = inp.tile([128, 3 * H], f32r, tag=f"wh{k}", name=f"wh{k}")
                nc.scalar.dma_start(out=t[:], in_=wh_d[r0:r1, :])
                wh_s.append(t)
                t = inp.tile([128, H], f32r, tag=f"whf{k}", name=f"whf{k}")
                nc.scalar.dma_start(out=t[:], in_=whf_d[r0:r1, :])
                whf_s.append(t)
            for k, (r0, r1) in enumerate(krows[:2] + [(256, 304)]):
                t = inp.tile([r1 - r0, XCOLS], bf16, tag=f"xt{k}", name=f"xt{k}")
                xt_s.append(t)
            # xt on sync, range-major so the first level's columns land first
            for (a, b) in RANGES:
                for k, (r0, r1) in enumerate(krows[:2] + [(256, 304)]):
                    nc.sync.dma_start(out=xt_s[k][:, a:b], in_=xt_d[r0:r1, a:b])

            # --- persistent state + gx tiles
            SH = {lv: st.tile([128, 2, n], f32r, tag=f"h{lv}", name=f"sh{lv}")
                  for lv, n in _STATE_COLS.items()}
            SC = {lv: st.tile([128, 2, n], f32, tag=f"c{lv}", name=f"sc{lv}")
                  for lv, n in _STATE_COLS.items()}
            GX = [st.tile([128, 2, XCOLS], f32, tag=f"gx{g}", name=f"gx{g}")
                  for g in range(4)]

            # --- phase 1: all x-side gate projections in one weight-stationary
            # sweep (minimal LDWEIGHTS: 24 distinct weight tiles, loaded once),
            # drained PSUM -> SBUF by DMA.
            for m in range(8):
                g, phi = divmod(m, 2)
                col = H * g + 128 * phi
                ptiles = [
                    ps.tile([128, b - a], f32, tag="gx", name=f"pgx{m}_{i}",
                            bufs=4)
                    for i, (a, b) in enumerate(RANGES)
                ]
                for k in range(3):
                    r0, r1 = krows[k]
                    for i, (a, b) in enumerate(RANGES):
                        mm(ptiles[i][:],
                           wx_s[k][0:r1 - r0, col:col + 128],
                           xt_s[k][0:r1 - r0, a:b],
                           start=(k == 0), stop=(k == 2))
                for i, (a, b) in enumerate(RANGES):
                    if (m + i) % 2 == 0:
                        nc.scalar.copy(GX[g][:, phi, a:b], ptiles[i][:])
                    else:
                        nc.vector.tensor_copy(GX[g][:, phi, a:b], ptiles[i][:])

            # --- phase 2: levels bottom-up in chunks
            for (xoff, cn, ip, child, coff, outlv, ooff) in _CHUNKS:
                hs = None
                if ip > 0:
                    # hs = sum of the 4 child h columns per node (on the
                    # otherwise-idle gpsimd engine)
                    hs = wk.tile([128, 2, ip], f32r, tag="hs", name="hs")
                    for phi in range(2):
                        cv = SH[child][:, phi, coff:coff + 4 * ip].rearrange(
                            "p (n k) -> p n k", k=K)
                        dst = hs[:, phi, :]
                        nc.gpsimd.tensor_add(dst, cv[:, :, 0], cv[:, :, 1])
                        nc.gpsimd.tensor_add(dst, dst, cv[:, :, 2])
                        nc.gpsimd.tensor_add(dst, dst, cv[:, :, 3])

                gates = []
                for gi_, func in ((0, Sig), (1, Sig), (2, Tanh)):
                    G = wk.tile([128, 2, cn], f32, tag=f"g{gi_}", name=f"g{gi_}")
                    if ip > 0:
                        P = ps.tile([128, 2, ip], f32, tag="pa", name=f"pa{gi_}",
                                    bufs=2)
                        for phi in range(2):
                            for k in range(2):
                                mm(P[:, phi, :],
                                   wh_s[k][:, H * gi_ + 128 * phi:
                                           H * gi_ + 128 * phi + 128],
                                   hs[:, k, 0:ip],
                                   start=(k == 0), stop=(k == 1))
                        nc.vector.tensor_add(
                            P[:], P[:], GX[gi_][:, :, xoff:xoff + ip])
                        nc.scalar.activation(G[:, :, 0:ip], P[:], func)
                        if cn > ip:
                            nc.scalar.activation(
                                G[:, :, ip:cn],
                                GX[gi_][:, :, xoff + ip:xoff + cn], func)
                    else:
                        nc.scalar.activation(
                            G[:], GX[gi_][:, :, xoff:xoff + cn], func)
                    gates.append(G)
                IG, OG, UG = gates

                csum = None
                if ip > 0:
                    # f = sigmoid(gf + Whf @ h_child), per child; then
                    # csum = sum_k f_k * c_child_k
                    csum = wk.tile([128, 2, ip], f32, tag="csum", name="csum")
                    for phi in range(2):
                        Pfc = ps.tile([128, 4 * ip], f32, tag="pf",
                                      name=f"pfc{phi}")
                        for k in range(2):
                            mm(Pfc[:],
                               whf_s[k][:, 128 * phi:128 * phi + 128],
                               SH[child][:, k, coff:coff + 4 * ip],
                               start=(k == 0), stop=(k == 1))
                        pv = Pfc.rearrange("p (n k) -> p n k", k=K)
                        gfb = GX[3][:, phi, xoff:xoff + ip][:, :, None]
                        nc.vector.tensor_add(
                            pv, pv, gfb.broadcast_to([128, ip, K]))
                        FS = fwk.tile([128, 4 * ip], f32, tag="fs", name="fs")
                        nc.scalar.activation(FS[:], Pfc[:], Sig)
                        nc.vector.tensor_mul(
                            FS[:], FS[:], SC[child][:, phi, coff:coff + 4 * ip])
                        sv = FS.rearrange("p (n k) -> p n k", k=K)
                        dst = csum[:, phi, :]
                        nc.gpsimd.tensor_add(dst, sv[:, :, 0], sv[:, :, 1])
                        nc.gpsimd.tensor_add(dst, dst, sv[:, :, 2])
                        nc.gpsimd.tensor_add(dst, dst, sv[:, :, 3])

                # c = ig*ug (+ csum on internal cols); h = og*tanh(c)
                Cdst = SC[outlv][:, :, ooff:ooff + cn]
                nc.vector.tensor_mul(Cdst, IG[:], UG[:])
                if ip > 0:
                    nc.vector.tensor_add(
                        SC[outlv][:, :, ooff:ooff + ip],
                        SC[outlv][:, :, ooff:ooff + ip],
                        csum[:],
                    )
                TC = wk.tile([128, 2, cn], f32, tag="tc", name="tc")
                nc.scalar.activation(TC[:], Cdst, Tanh)
                nc.vector.tensor_mul(
                    SH[outlv][:, :, ooff:ooff + cn], OG[:], TC[:])

            nc.sync.dma_start(out=out_h_d[:], in_=SH[4][:])
            nc.sync.dma_start(out=out_c_d[:], in_=SC[4][:])

    nc.compile()
    return nc


def _get_compiled():
    global _COMPILED
    if _COMPILED is None:
        _COMPILED = _build_device_program()
    return _COMPILED


def _numpy_fallback(xs, child_idx, child_mask, emb_table, Wx, bx, Wh, bh,
                    Wout, bout):
    """Exact sequential scan in numpy; only used if the tree is not the
    expected static 4-ary heap."""
    X = np.asarray(emb_table, dtype=F32)[np.asarray(xs)]
    Wx = np.asarray(Wx, dtype=F32)
    Wh = np.asarray(Wh, dtype=F32)
    bx = np.asarray(bx, dtype=F32)
    bh = np.asarray(bh, dtype=F32)
    gx = np.einsum('ghd,nd->ngh', Wx, X).astype(F32) + bx
    Hb = np.zeros((N, H), dtype=F32)
    Cb = np.zeros((N, H), dtype=F32)
    ci = np.asarray(child_idx)
    cm = np.asarray(child_mask, dtype=F32)
    for i in range(N - 1, -1, -1):
        idx = ci[i]
        m = cm[i][:, None]
        Hc = Hb[idx] * m
        Cc = Cb[idx] * m
        hs = Hc.sum(0)
        g = gx[i]
        ig = _sigmoid(g[0] + Wh[0] @ hs + bh[0])
        og = _sigmoid(g[2] + Wh[2] @ hs + bh[2])
        ug = np.tanh(g[3] + Wh[3] @ hs + bh[3]).astype(F32)
        f = _sigmoid(g[1] + Hc @ Wh[1].T + bh[1])
        c = ig * ug + (f * Cc).sum(0)
        Hb[i] = og * np.tanh(c).astype(F32)
        Cb[i] = c
    logits = np.asarray(Wout, dtype=F32) @ Hb[0] + np.asarray(bout, dtype=F32)
    return _log_softmax(logits)


def kernel(xs, child_idx, child_mask, emb_table, Wx, bx, Wh, bh, Wout, bout):
    xs = np.asarray(xs)
    if not (np.array_equal(np.asarray(child_idx), _STATIC_IDX)
            and np.array_equal(np.asarray(child_mask, dtype=F32), _STATIC_MASK)):
        return _numpy_fallback(xs, child_idx, child_mask, emb_table, Wx, bx,
                               Wh, bh, Wout, bout)

    from concourse.bass_utils import run_bass_kernel_spmd

    wx, wh, whf = _pack_weights(Wx, bx, Wh, bh)
    xts = _pack_xt(xs, emb_table)
    in_maps = [
        {"xt": xts[c], "wx": wx, "wh": wh, "whf": whf} for c in range(NCORES)
    ]
    nc = _get_compiled()
    res = run_bass_kernel_spmd(nc, in_maps, core_ids=list(range(NCORES)))

    Hbuf = np.zeros((341, H), dtype=F32)
    Cbuf = np.zeros((341, H), dtype=F32)
    for c in range(NCORES):
        _, _, l4 = _PLAN[c]
        oh = res.results[c]["out_h"]   # [128, 2, 32]
        oc = res.results[c]["out_c"]
        Hbuf[l4] = np.concatenate([oh[:, 0, :], oh[:, 1, :]], axis=0).T
        Cbuf[l4] = np.concatenate([oc[:, 0, :], oc[:, 1, :]], axis=0).T

    h0 = _host_top(Hbuf, Cbuf, xs, emb_table, Wx, bx, Wh, bh)
    logits = np.asarray(Wout, dtype=F32) @ h0 + np.asarray(bout, dtype=F32)
    return _log_softmax(logits)



# revision 16
# speedup vs baseline: 1.2594x; 1.2594x over previous
"""ChildSum TreeLSTM (N=8192 nodes, 4-ary static heap tree, H=256, D=300) on 8 trn2 NeuronCores.

Strategy
--------
The tree is static: node i's children are 4i+1..4i+4 (clipped at N). The reverse
scan (children before parents) is equivalent to processing the tree level by
level, bottom-up; nodes within a level are independent, so each level is a
batched LSTM cell (matmuls + elementwise).

Sharding: the 256 level-4 subtrees are partitioned across the 8 cores (balanced
by the number of *internal* level-6 descendants). Each core processes its
forest fully locally; the tiny top of the tree (levels 3..0, 85 nodes) plus the
final log_softmax run on the host in numpy.

On-device layout: feature dim on SBUF partitions (256 features = 2 halves of
128), nodes along the free axis, bf16 operands for all matmuls.

Column order per core: [L7 leaves (384) | L6 leaf-only (416) | L6 internal (96)
| L5 (128) | L4 (32)]. Leaf columns (0..800) get their gates computed by a
weight-stationary x-side sweep whose PSUM is drained directly through the
sigmoid/tanh activations (no gx staging). Internal columns (800..1056) keep
their x-side gate pre-activations resident in PSUM; each level's h-side matmuls
accumulate into the same PSUM region, so gates come out of a single activation
with no extra adds. The f-gate x-projection is only computed for the 256
internal columns (leaf forget gates are unused).
"""

import numpy as np
import ml_dtypes

BF16 = ml_dtypes.bfloat16

N = 8192
H = 256
D = 300
K = 4
OUT = 4
NCORES = 8
IPMAX = 96          # max internal level-6 nodes per core
L7P = 384           # padded level-7 columns per core (4 * IPMAX)
L6N = 512
L6LEAF = L6N - IPMAX    # 416
LEAFC = L7P + L6LEAF    # 800 leaf columns
INTC = IPMAX + 128 + 32  # 256 internal columns
XCOLS = LEAFC + INTC     # 1056
KDIM = 304          # padded contraction rows (300 emb + 1 ones + 3 pad)

GATE_MAP = [0, 2, 3, 1]  # our gate order [i, o, u, f] -> reference gate indices

F32 = np.float32


def _build_plan():
    """Assign the 256 level-4 subtrees to 8 cores; build per-core column maps."""
    full = list(range(85, 127))                               # 42 subtrees, w=16
    lights = list(range(128, 341))                            # 213 subtrees, w=0
    heavy_counts = [6, 6, 5, 5, 5, 5, 5, 5]                   # sums to 42
    light_counts = [26, 26, 26, 27, 27, 27, 27, 27]           # sums to 213
    cores = []
    hpos = 0
    lpos = 0
    for c in range(NCORES):
        hs = full[hpos:hpos + heavy_counts[c]]
        hpos += heavy_counts[c]
        if c == 2:
            hs = hs + [127]                                   # w sums: 96,96,91,80*5
        ls = lights[lpos:lpos + light_counts[c]]
        lpos += light_counts[c]
        cores.append(sorted(hs + ls))
    all_l4 = sorted(u for cs in cores for u in cs)
    assert all_l4 == list(range(85, 341)), "L4 assignment must partition [85, 341)"

    plan = []
    for c in range(NCORES):
        l4 = cores[c]
        assert len(l4) == 32
        l5 = [4 * u + 1 + k for u in l4 for k in range(K)]
        l6 = [4 * v + 1 + k for v in l5 for k in range(K)]
        wc = sum(1 for x in l6 if x < 2048)
        assert wc <= IPMAX
        # heavy l4 sort first, so internal l6 nodes are exactly l6[:wc]
        assert all(x < 2048 for x in l6[:wc])
        l7 = []
        for x in l6[:IPMAX]:
            for k in range(K):
                ch = 4 * x + 1 + k
                l7.append(ch if ch < N else -1)
        assert len(l7) == L7P
        # xt column order: [L7 | L6 leaf part | L6 internal part | L5 | L4]
        cols = np.array(l7 + l6[IPMAX:] + l6[:IPMAX] + l5 + l4, dtype=np.int64)
        assert cols.shape == (XCOLS,)
        plan.append((cols, wc, np.array(l4, dtype=np.int64)))
    return plan


_PLAN = _build_plan()

# chunk schedule: (ip, child_level, out_level, out_off, delta)
#   delta = offset of this chunk's columns inside the internal region
#   chunk columns in xt: LEAFC+delta .. LEAFC+delta+ip
_CHUNKS = [
    (IPMAX, 7, 6, 0, 0),        # L6 internal   (children: all 384 L7 cols)
    (128,   6, 5, 0, IPMAX),    # L5            (children: all 512 L6 cols)
    (32,    5, 4, 0, IPMAX + 128),  # L4        (children: all 128 L5 cols)
]
_STATE_COLS = {7: L7P, 6: L6N, 5: 128, 4: 32}


def _static_tree():
    idx = np.arange(N)[:, None] * K + 1 + np.arange(K)[None, :]
    mask = (idx < N).astype(F32)
    idx = np.where(idx < N, idx, 0).astype(np.int32)
    return idx, mask


_STATIC_IDX, _STATIC_MASK = _static_tree()


def _pack_weights(Wx, bx, Wh, bh):
    # wx: [KDIM, 1024] bf16, gate blocks [i|o|u|f] each 256 wide ([phi0|phi1])
    wx = np.zeros((KDIM, 4 * H), dtype=F32)
    for g, rg in enumerate(GATE_MAP):
        wx[:D, H * g:H * (g + 1)] = np.asarray(Wx[rg], dtype=F32).T
        wx[D, H * g:H * (g + 1)] = np.asarray(bx[rg], dtype=F32) + np.asarray(bh[rg], dtype=F32)
    # whh: [2, 128, 1024] bf16; k = input-feature half on partitions,
    # cols = [i|o|u|f] gate blocks of 256 ([phi0|phi1] out-feature halves)
    whh = np.zeros((2, 128, 4 * H), dtype=F32)
    for g, rg in enumerate(GATE_MAP):
        wT = np.asarray(Wh[rg], dtype=F32).T        # [in, out]
        for k in range(2):
            whh[k, :, H * g:H * (g + 1)] = wT[128 * k:128 * (k + 1), :]
    return wx.astype(BF16), whh.astype(BF16)


def _pack_xt(xs, emb_table):
    X = np.asarray(emb_table, dtype=F32)[np.asarray(xs)]
    xts = []
    for cols, _, _ in _PLAN:
        xt = np.zeros((KDIM, XCOLS), dtype=F32)
        real = cols >= 0
        xt[:D, real] = X[cols[real]].T
        xt[D, real] = 1.0
        xts.append(xt.astype(BF16))
    return xts


def _sigmoid(x):
    return (1.0 / (1.0 + np.exp(-x))).astype(F32)


def _host_top(Hbuf, Cbuf, xs, emb_table, Wx, bx, Wh, bh):
    """Compute tree levels 3..0 (nodes 0..84) on the host, numpy fp32."""
    Wx = np.asarray(Wx, dtype=F32)
    bx = np.asarray(bx, dtype=F32)
    Wh = np.asarray(Wh, dtype=F32)
    bh = np.asarray(bh, dtype=F32)
    emb = np.asarray(emb_table, dtype=F32)
    xs = np.asarray(xs)
    for lo, hi in [(21, 85), (5, 21), (1, 5), (0, 1)]:
        ids = np.arange(lo, hi)
        Xl = emb[xs[ids]]                                   # [n, D]
        gx = np.einsum('ghd,nd->ngh', Wx, Xl).astype(F32) + bx
        cidx = ids[:, None] * K + 1 + np.arange(K)[None, :]  # all valid (< 341)
        Hc = Hbuf[cidx]
        Cc = Cbuf[cidx]
        hs = Hc.sum(1)
        ig = _sigmoid(gx[:, 0] + hs @ Wh[0].T + bh[0])
        og = _sigmoid(gx[:, 2] + hs @ Wh[2].T + bh[2])
        ug = np.tanh(gx[:, 3] + hs @ Wh[3].T + bh[3]).astype(F32)
        f = _sigmoid(gx[:, 1][:, None, :] + Hc @ Wh[1].T + bh[1])
        cc = ig * ug + (f * Cc).sum(1)
        hh = og * np.tanh(cc).astype(F32)
        Hbuf[ids] = hh
        Cbuf[ids] = cc
    return Hbuf[0]


def _log_softmax(x):
    m = np.max(x)
    e = np.exp(x - m)
    return (x - m - np.log(e.sum())).astype(F32)


def simulate_cores_numpy(inputs):
    """Numpy emulation of the device data layout & schedule (for plan checks).

    Returns (Hbuf, Cbuf) filled for nodes [85, 341).
    """
    xs = np.asarray(inputs["xs"])
    wx, whh = _pack_weights(inputs["Wx"], inputs["bx"], inputs["Wh"], inputs["bh"])
    xts = _pack_xt(xs, inputs["emb_table"])
    wxf = wx.astype(F32)
    whf = whh.astype(F32)   # [2, 128, 1024]
    whcat = np.concatenate([whf[0], whf[1]], axis=0)  # [256, 1024]
    Hbuf = np.zeros((341, H), dtype=F32)
    Cbuf = np.zeros((341, H), dtype=F32)
    for c in range(NCORES):
        cols, wc, l4 = _PLAN[c]
        xt = xts[c].astype(F32)
        # phase A: leaf sweep, gates i/o/u over cols [0, LEAFC)
        gl = wxf[:, :3 * H].T @ xt[:, :LEAFC]          # [768, 800]
        IGl = _sigmoid(gl[0:H])
        OGl = _sigmoid(gl[H:2 * H])
        UGl = np.tanh(gl[2 * H:3 * H]).astype(F32)
        Cl = (IGl * UGl).astype(F32)
        Hl = (OGl * np.tanh(Cl)).astype(F32)
        # int sweep: all four gates over cols [LEAFC, XCOLS)
        gint = wxf.T @ xt[:, LEAFC:]                    # [1024, 256]
        state_h = {lv: np.zeros((H, n), dtype=F32) for lv, n in _STATE_COLS.items()}
        state_c = {lv: np.zeros((H, n), dtype=F32) for lv, n in _STATE_COLS.items()}
        state_h[7] = Hl[:, :L7P]
        state_c[7] = Cl[:, :L7P]
        state_h[6][:, IPMAX:] = Hl[:, L7P:]
        state_c[6][:, IPMAX:] = Cl[:, L7P:]
        for (ip, child, outlv, ooff, delta) in _CHUNKS:
            ch_h = state_h[child][:, :4 * ip].astype(BF16).astype(F32)
            ch_c = state_c[child][:, :4 * ip]
            hs = ch_h.reshape(H, ip, K).sum(axis=2).astype(BF16).astype(F32)
            g = gint[:, delta:delta + ip].copy()        # [1024, ip]
            A = whcat[:, :3 * H].T @ hs                 # [768, ip]
            g[:3 * H] += A
            Fp = whcat[:, 3 * H:].T @ ch_h              # [256, 4ip]
            FA = Fp + np.repeat(g[3 * H:], K, axis=1)
            FS = _sigmoid(FA).astype(BF16).astype(F32) * ch_c
            csum = FS.reshape(H, ip, K).sum(axis=2)
            ig = _sigmoid(g[0:H])
            og = _sigmoid(g[H:2 * H])
            ug = np.tanh(g[2 * H:3 * H]).astype(F32)
            cc = (ig * ug + csum).astype(F32)
            hh = (og * np.tanh(cc)).astype(F32)
            state_h[outlv][:, ooff:ooff + ip] = hh
            state_c[outlv][:, ooff:ooff + ip] = cc
        Hbuf[l4] = state_h[4].T
        Cbuf[l4] = state_c[4].T
    return Hbuf, Cbuf


# ----------------------------------------------------------------------------
# Bass device program
# ----------------------------------------------------------------------------

_COMPILED = None


def _build_device_program():
    import contextlib

    import concourse.bacc as bacc
    import concourse.tile as tile
    import concourse.mybir as mybir

    f32 = mybir.dt.float32
    bf16 = mybir.dt.bfloat16
    Sig = mybir.ActivationFunctionType.Sigmoid
    Tanh = mybir.ActivationFunctionType.Tanh

    nc = bacc.Bacc("TRN2", target_bir_lowering=False, debug=False,
                   num_devices=NCORES)

    xt_d = nc.dram_tensor("xt", [KDIM, XCOLS], bf16, kind="ExternalInput")
    wx_d = nc.dram_tensor("wx", [KDIM, 4 * H], bf16, kind="ExternalInput")
    whh_d = nc.dram_tensor("whh", [2, 128, 4 * H], bf16, kind="ExternalInput")
    out_h_d = nc.dram_tensor("out_h", [128, 2, 32], f32, kind="ExternalOutput")
    out_c_d = nc.dram_tensor("out_c", [128, 2, 32], f32, kind="ExternalOutput")

    KR = [(0, 128), (128, 256), (256, KDIM)]

    with tile.TileContext(nc) as tc:
        with contextlib.ExitStack() as ctx:
            inp = ctx.enter_context(tc.tile_pool(name="inp", bufs=1))
            st = ctx.enter_context(tc.tile_pool(name="state", bufs=1))
            wk = ctx.enter_context(tc.tile_pool(name="work", bufs=2))
            ps = ctx.enter_context(tc.tile_pool(name="psum", bufs=2, space="PSUM"))
            psi = ctx.enter_context(tc.tile_pool(name="psint", bufs=1, space="PSUM"))

            # ---- input DMAs, batched, spread across queues --------------
            xt_s = []
            for k, (r0, r1) in enumerate(KR):
                xt_s.append(inp.tile([r1 - r0, XCOLS], bf16, tag=f"xt{k}",
                                     name=f"xt{k}"))
            # leaf columns first (sync + scalar queues in parallel)
            nc.sync.dma_start(out=xt_s[0][:, 0:LEAFC], in_=xt_d[0:128, 0:LEAFC])
            nc.scalar.dma_start(out=xt_s[2][:, 0:LEAFC],
                                in_=xt_d[256:KDIM, 0:LEAFC])
            nc.sync.dma_start(out=xt_s[1][:, 0:LEAFC], in_=xt_d[128:256, 0:LEAFC])
            # internal columns next
            nc.sync.dma_start(out=xt_s[0][:, LEAFC:], in_=xt_d[0:128, LEAFC:])
            nc.scalar.dma_start(out=xt_s[2][:, LEAFC:], in_=xt_d[256:KDIM, LEAFC:])
            nc.sync.dma_start(out=xt_s[1][:, LEAFC:], in_=xt_d[128:256, LEAFC:])

            wx_s = []
            for k, (r0, r1) in enumerate(KR):
                wx_s.append(inp.tile([r1 - r0, 4 * H], bf16, tag=f"wx{k}",
                                     name=f"wx{k}"))
            # first half [i|o], then [u|f] — matches hg sweep order i,o,u
            for k, (r0, r1) in enumerate(KR):
                nc.gpsimd.dma_start(out=wx_s[k][:, 0:512], in_=wx_d[r0:r1, 0:512])
            for k, (r0, r1) in enumerate(KR):
                nc.gpsimd.dma_start(out=wx_s[k][:, 512:1024],
                                    in_=wx_d[r0:r1, 512:1024])

            whh_s = []
            for k in range(2):
                t = inp.tile([128, 4 * H], bf16, tag=f"whh{k}", name=f"whh{k}")
                nc.sync.dma_start(out=t[:], in_=whh_d[k])
                whh_s.append(t)

            # ---- persistent state ---------------------------------------
            SH = {lv: st.tile([128, 2, n], bf16, tag=f"h{lv}", name=f"sh{lv}")
                  for lv, n in _STATE_COLS.items() if lv > 4}
            SC = {lv: st.tile([128, 2, n], f32, tag=f"c{lv}", name=f"sc{lv}")
                  for lv, n in _STATE_COLS.items() if lv > 4}
            OH = st.tile([128, 2, 32], f32, tag="oh", name="oh")
            OC = st.tile([128, 2, 32], f32, tag="oc", name="oc")

            # leaf gate tiles (drain targets of the x-sweep)
            GL = {}
            for gname in ("i", "o", "u"):
                GL[gname] = st.tile([128, 2, LEAFC], bf16, tag=f"gl{gname}",
                                    name=f"gl{gname}")
            TCL = st.tile([128, 2, LEAFC], bf16, tag="tcl", name="tcl")
            GF = st.tile([128, 2, INTC], f32, tag="gf", name="gf")
            GXO = st.tile([128, 2, INTC], f32, tag="gxo", name="gxo")

            # ---- phase A1: leaf x-sweep, fused PSUM->activation drain ---
            # hg order: i,o,u per phi; activations consume PSUM directly.
            # matmul outputs must stay within one PSUM bank (512 f32 cols).
            leaf_act = {"i": Sig, "o": Sig, "u": Tanh}
            LRANGES = [(0, 512), (512, LEAFC)]
            for gi_, gname in enumerate(("i", "o", "u")):
                for phi in range(2):
                    col = 256 * gi_ + 128 * phi
                    for (a, b) in LRANGES:
                        P = ps.tile([128, b - a], f32, tag="lp",
                                    name=f"lp{gname}{phi}{a}", bufs=2)
                        for k, (r0, r1) in enumerate(KR):
                            nc.tensor.matmul(P[:], wx_s[k][:, col:col + 128],
                                             xt_s[k][:, a:b],
                                             start=(k == 0), stop=(k == 2))
                        nc.scalar.activation(GL[gname][:, phi, a:b], P[:],
                                             leaf_act[gname])

            # ---- phase A2: internal x-sweep ------------------------------
            # i/u pre-activations stay PSUM-resident (one bank each, both
            # phi); the per-level h-side matmuls accumulate into them later.
            # o and f drain to SBUF (GXO / GF) to stay within 8 PSUM banks.
            # NOTE: a start=True matmul marks its whole 2KB PSUM bank as
            # pending-zero, so each resident accumulator must own its bank
            # (exactly one start=True per bank, at k==0 of the x-sweep).
            PI = {}
            for gname, gi_ in (("i", 0), ("u", 2)):
                for phi in range(2):
                    P = psi.tile([128, INTC], f32, tag=f"pi{gname}{phi}",
                                 name=f"pi{gname}{phi}")
                    PI[(gname, phi)] = P
                    col = 256 * gi_ + 128 * phi
                    for k, (r0, r1) in enumerate(KR):
                        nc.tensor.matmul(P[:], wx_s[k][:, col:col + 128],
                                         xt_s[k][:, LEAFC:],
                                         start=(k == 0), stop=(k == 2),
                                         skip_group_check=True)
            for gname, gi_, dst in (("o", 1, GXO), ("f", 3, GF)):
                for phi in range(2):
                    col = 256 * gi_ + 128 * phi
                    P = ps.tile([128, INTC], f32, tag="lp", name=f"px{gname}{phi}",
                                bufs=2)
                    for k, (r0, r1) in enumerate(KR):
                        nc.tensor.matmul(P[:], wx_s[k][:, col:col + 128],
                                         xt_s[k][:, LEAFC:],
                                         start=(k == 0), stop=(k == 2))
                    nc.scalar.copy(dst[:, phi, :], P[:])

            # ---- leaf c/h (phi0 on vector, phi1 on gpsimd) --------------
            # c = ig*ug -> SC7 / SC6[leaf];  h = og*tanh(c) -> SH7 / SH6[leaf]
            for phi, eng in ((0, nc.vector), (1, nc.gpsimd)):
                eng.tensor_mul(SC[7][:, phi, :],
                               GL["i"][:, phi, 0:L7P], GL["u"][:, phi, 0:L7P])
                eng.tensor_mul(SC[6][:, phi, IPMAX:],
                               GL["i"][:, phi, L7P:], GL["u"][:, phi, L7P:])
            nc.scalar.activation(TCL[:, :, 0:L7P], SC[7][:], Tanh)
            nc.scalar.activation(TCL[:, :, L7P:], SC[6][:, :, IPMAX:], Tanh)
            for phi, eng in ((0, nc.vector), (1, nc.gpsimd)):
                eng.tensor_mul(SH[7][:, phi, :],
                               GL["o"][:, phi, 0:L7P], TCL[:, phi, 0:L7P])
                eng.tensor_mul(SH[6][:, phi, IPMAX:],
                               GL["o"][:, phi, L7P:], TCL[:, phi, L7P:])

            # ---- phase B: internal chunks bottom-up ---------------------
            for ci, (ip, child, outlv, ooff, delta) in enumerate(_CHUNKS):
                ch_h = SH[child]
                ch_c = SC[child]
                last = (outlv == 4)

                # f-side matmuls first (they don't need hs); per-phi one-bank
                # PSUM tiles shared with the leaf-sweep ring via tag "pfc"
                FS = wk.tile([128, 2, 4 * ip], bf16, tag="fs", name=f"fs{ci}")
                FSC = wk.tile([128, 2, 4 * ip], f32, tag="fsc", name=f"fsc{ci}")
                csum = wk.tile([128, 2, ip], f32, tag="csum", name=f"csum{ci}")
                for phi in range(2):
                    PF = ps.tile([128, 4 * ip], f32, tag="pfc",
                                 name=f"pfc{ci}{phi}", bufs=2)
                    for k in range(2):
                        nc.tensor.matmul(PF[:],
                                         whh_s[k][:, 768 + 128 * phi:
                                                  768 + 128 * phi + 128],
                                         ch_h[:, k, 0:4 * ip],
                                         start=(k == 0), stop=(k == 1))
                    pv = PF.rearrange("p (n k) -> p n k", k=K)
                    gfb = GF[:, phi, delta:delta + ip][:, :, None]
                    nc.vector.tensor_add(pv, pv, gfb.broadcast_to([128, ip, K]))
                    nc.scalar.activation(FS[:, phi, :], PF[:], Sig)

                # child h sums on gpsimd (overlaps the f matmuls)
                hs = wk.tile([128, 2, ip], bf16, tag="hs", name=f"hs{ci}")
                for phi in range(2):
                    cv = ch_h[:, phi, 0:4 * ip].rearrange("p (n k) -> p n k", k=K)
                    dst = hs[:, phi, :]
                    nc.gpsimd.tensor_add(dst, cv[:, :, 0], cv[:, :, 1])
                    nc.gpsimd.tensor_add(dst, dst, cv[:, :, 2])
                    nc.gpsimd.tensor_add(dst, dst, cv[:, :, 3])

                # i/u h-side matmuls accumulate into the resident x-side PSUM
                for gname, gi_ in (("i", 0), ("u", 2)):
                    for phi in range(2):
                        col = 256 * gi_ + 128 * phi
                        for k in range(2):
                            nc.tensor.matmul(PI[(gname, phi)][:, delta:delta + ip],
                                             whh_s[k][:, col:col + 128],
                                             hs[:, k, :],
                                             start=False,
                                             stop=(k == 1),
                                             skip_group_check=True)

                # f-path: multiply by child c, reduce over the 4 children
                nc.vector.tensor_mul(FSC[:], FS[:], ch_c[:, :, 0:4 * ip])
                for phi in range(2):
                    fv = FSC[:, phi, :].rearrange("p (n k) -> p n k", k=K)
                    nc.vector.tensor_reduce(csum[:, phi, :], fv,
                                            mybir.AxisListType.X,
                                            mybir.AluOpType.add)

                # gates: i/u straight from resident PSUM; o via fresh PSUM + GXO
                G2 = {g: wk.tile([128, 2, ip], bf16, tag=f"g2{g}",
                                 name=f"g2{g}{ci}") for g in ("i", "o", "u")}
                for gname, func in (("i", Sig), ("u", Tanh)):
                    for phi in range(2):
                        nc.scalar.activation(G2[gname][:, phi, :],
                                             PI[(gname, phi)][:, delta:delta + ip],
                                             func)
                for phi in range(2):
                    PO = ps.tile([128, ip], f32, tag="pfc", name=f"po{ci}{phi}",
                                 bufs=2)
                    for k in range(2):
                        nc.tensor.matmul(PO[:],
                                         whh_s[k][:, 256 + 128 * phi:
                                                  256 + 128 * phi + 128],
                                         hs[:, k, :],
                                         start=(k == 0), stop=(k == 1))
                    nc.vector.tensor_add(PO[:], PO[:],
                                         GXO[:, phi, delta:delta + ip])
                    nc.scalar.activation(G2["o"][:, phi, :], PO[:], Sig)
                if last:
                    Cdst = OC[:]
                    Hdst = OH[:]
                else:
                    Cdst = SC[outlv][:, :, ooff:ooff + ip]
                    Hdst = SH[outlv][:, :, ooff:ooff + ip]
                nc.vector.tensor_mul(Cdst, G2["i"][:], G2["u"][:])
                nc.vector.tensor_add(Cdst, Cdst, csum[:])
                TC2 = wk.tile([128, 2, ip], bf16, tag="tc2", name=f"tc2{ci}")
                nc.scalar.activation(TC2[:], Cdst, Tanh)
                nc.vector.tensor_mul(Hdst, G2["o"][:], TC2[:])

            nc.sync.dma_start(out=out_h_d[:], in_=OH[:])
            nc.sync.dma_start(out=out_c_d[:], in_=OC[:])

    nc.compile()
    return nc


def _get_compiled():
    global _COMPILED
    if _COMPILED is None:
        _COMPILED = _build_device_program()
    return _COMPILED


def _numpy_fallback(xs, child_idx, child_mask, emb_table, Wx, bx, Wh, bh,
                    Wout, bout):
    """Exact sequential scan in numpy; only used if the tree is not the
    expected static 4-ary heap."""
    X = np.asarray(emb_table, dtype=F32)[np.asarray(xs)]
    Wx = np.asarray(Wx, dtype=F32)
    Wh = np.asarray(Wh, dtype=F32)
    bx = np.asarray(bx, dtype=F32)
    bh = np.asarray(bh, dtype=F32)
    gx = np.einsum('ghd,nd->ngh', Wx, X).astype(F32) + bx
    Hb = np.zeros((N, H), dtype=F32)
    Cb = np.zeros((N, H), dtype=F32)
    ci = np.asarray(child_idx)
    cm = np.asarray(child_mask, dtype=F32)
    for i in range(N - 1, -1, -1):
        idx = ci[i]
        m = cm[i][:, None]
        Hc = Hb[idx] * m
        Cc = Cb[idx] * m
        hs = Hc.sum(0)
        g = gx[i]
        ig = _sigmoid(g[0] + Wh[0] @ hs + bh[0])
        og = _sigmoid(g[2] + Wh[2] @ hs + bh[2])
        ug = np.tanh(g[3] + Wh[3] @ hs + bh[3]).astype(F32)
        f = _sigmoid(g[1] + Hc @ Wh[1].T + bh[1])
        c = ig * ug + (f * Cc).sum(0)
        Hb[i] = og * np.tanh(c).astype(F32)
        Cb[i] = c
    logits = np.asarray(Wout, dtype=F32) @ Hb[0] + np.asarray(bout, dtype=F32)
    return _log_softmax(logits)


def kernel(xs, child_idx, child_mask, emb_table, Wx, bx, Wh, bh, Wout, bout):
    xs = np.asarray(xs)
    if not (np.array_equal(np.asarray(child_idx), _STATIC_IDX)
            and np.array_equal(np.asarray(child_mask, dtype=F32), _STATIC_MASK)):
        return _numpy_fallback(xs, child_idx, child_mask, emb_table, Wx, bx,
                               Wh, bh, Wout, bout)

    from concourse.bass_utils import run_bass_kernel_spmd

    wx, whh = _pack_weights(Wx, bx, Wh, bh)
    xts = _pack_xt(xs, emb_table)
    in_maps = [
        {"xt": xts[c], "wx": wx, "whh": whh} for c in range(NCORES)
    ]
    nc = _get_compiled()
    res = run_bass_kernel_spmd(nc, in_maps, core_ids=list(range(NCORES)))

    Hbuf = np.zeros((341, H), dtype=F32)
    Cbuf = np.zeros((341, H), dtype=F32)
    for c in range(NCORES):
        _, _, l4 = _PLAN[c]
        oh = res.results[c]["out_h"]   # [128, 2, 32]
        oc = res.results[c]["out_c"]
        Hbuf[l4] = np.concatenate([oh[:, 0, :], oh[:, 1, :]], axis=0).T
        Cbuf[l4] = np.concatenate([oc[:, 0, :], oc[:, 1, :]], axis=0).T

    h0 = _host_top(Hbuf, Cbuf, xs, emb_table, Wx, bx, Wh, bh)
    logits = np.asarray(Wout, dtype=F32) @ h0 + np.asarray(bout, dtype=F32)
    return _log_softmax(logits)
